# revision 10
# baseline (speedup 1.0000x reference)
"""Trainium2 Bass kernel for nn_End2EndRVFixedOutput (nms_detection).

Reference semantics: out[100,7] starts at zeros; for n = 0..7 in order,
with off_n = (0 if n==0 else num_dets[n-1]) and k_n = num_dets[n],
rows [off_n, off_n+k_n) are overwritten with
[n, boxes[n,j,0:4], classes[n,j], scores[n,j]] for j = row-off_n.

num_dets < 12, so only the [:, :12] input slices matter and only out rows
0..21 can ever be written.  Device algorithm (per core, inputs replicated):

  1. x7[96,7] = [vd | boxes | classes | scores] for rows p = 12n+j is
     assembled by direct column DMAs straight from the full DRAM tensors.
  2. Winner masks are computed deterministically on DVE + PE:
        rm8[n,r]  = (off_n <= r < off_n+k_n)        # batch n covers row r
        stn[n,r]  = sum_{m>n} rm8[m,r]              # tiny suffix matmul
        effT[n,r] = rm8[n,r] * (stn[n,r]==0)        # n is the LAST writer
        EFF96     = SEL96 @ effT                    # broadcast to (n,j) rows
     Per-row scatter targets and winner gating:
        rpv[p]  = off_n + j + 1e6*(j >= k_n)
        w96[p]  = sum_r (R100[p,r]==rpv[p]) * EFF96[p,r]   # fused accum_out
        ridx[p] = rpv[p] + 1e6*(1 - w96[p])
  3. A zero-fill indirect DMA writes zeros to all 100 rows, then the data
     scatter writes x7 rows to out[ridx] on the same qPoolDynamic ring.
     Gating makes destinations UNIQUE (at most one winner per row), so no
     reliance on DMA descriptor ordering; indices >= 1e6 are skipped via
     bounds_check.

All arithmetic is exact in fp32 (masks are 0/1, indices are small ints),
so the output matches the reference bit-for-bit.  Every core runs the
full (tiny) computation; core 0's output is returned.
"""

import sys

import numpy as np

_TRN_REPO = "/opt/trn_rl_repo"
if _TRN_REPO not in sys.path:
    sys.path.insert(0, _TRN_REPO)

import concourse.bacc as bacc
import concourse.bass as bass
import concourse.mybir as mybir
import concourse.tile as tile
from concourse.bass_types import AP
from concourse.bass_utils import run_bass_kernel_spmd

B = 8          # batches
N_FULL = 8192  # detections per batch in the full input
J = 12         # num_dets < 12, so only rows [:12] of each batch matter
R = 100        # fixed output rows
P96 = B * J    # 96 stacked (batch, j) rows
OOB = 1.0e6    # pushed past bounds_check so the scatter skips the row

F32 = mybir.dt.float32
I32 = mybir.dt.int32

# Constant blob (float32): j96 [96] | vd96 [96] | R8 [8,100] | U8 [8,8] |
# SEL96 [8,96] | R100 [96,100]
_O_J96 = 0
_O_VD = _O_J96 + P96
_O_R8 = _O_VD + P96
_O_U8SEL = _O_R8 + 8 * R          # U8 and SEL96 packed as one [8,104] block
_O_R100 = _O_U8SEL + 8 * (8 + P96)
CONST_LEN = _O_R100 + P96 * R


def _make_consts() -> np.ndarray:
    p = np.arange(P96)
    m = np.arange(B)
    j96 = (p % J).astype(np.float32)
    vd96 = (p // J).astype(np.float32)
    r8 = np.tile(np.arange(R, dtype=np.float32)[None, :], (B, 1))        # [8,100]
    u8 = (m[:, None] > m[None, :]).astype(np.float32)                    # [8,8]
    sel96 = (m[:, None] == p[None, :] // J).astype(np.float32)           # [8,96]
    r8usel = np.concatenate([r8, u8, sel96], axis=1)                     # [8,204]
    r100 = np.tile(np.arange(R, dtype=np.float32)[None, :], (P96, 1))    # [96,100]
    blob = np.concatenate(
        [j96, vd96, r8usel.ravel(), r100.ravel()]
    ).astype(np.float32)
    assert blob.shape == (CONST_LEN,)
    return np.ascontiguousarray(blob)


def _build_nc() -> bass.Bass:
    nc = bacc.Bacc(None, target_bir_lowering=False)
    nd_d = nc.dram_tensor("num_dets", [B], I32, kind="ExternalInput")
    boxes_d = nc.dram_tensor("boxes", [B, N_FULL, 4], F32, kind="ExternalInput")
    scores_d = nc.dram_tensor("scores", [B, N_FULL], F32, kind="ExternalInput")
    classes_d = nc.dram_tensor("classes", [B, N_FULL], F32, kind="ExternalInput")
    const_d = nc.dram_tensor("consts", [CONST_LEN], F32, kind="ExternalInput")
    out_d = nc.dram_tensor("out", [R, 7], F32, kind="ExternalOutput")

    with tile.TileContext(nc) as tc:
        with (
            tc.tile_pool(name="sb", bufs=1) as sb,
            tc.tile_pool(name="ps", bufs=1, space=bass.MemorySpace.PSUM) as ps,
        ):
            k8i = sb.tile([B, 1], I32)
            off8i = sb.tile([B, 1], I32)
            k96 = sb.tile([P96, 1], I32)
            off96 = sb.tile([P96, 1], I32)
            j96 = sb.tile([P96, 1], F32)
            r8usel = sb.tile([B, R + 8 + P96], F32)   # R8 | U8 | SEL96
            r100 = sb.tile([P96, R], F32)
            x7 = sb.tile([P96, 7], F32)
            z7 = sb.tile([R, 7], F32)
            ridx0 = sb.tile([R, 1], I32)

            off8f = sb.tile([B, 1], F32)
            s8f = sb.tile([B, 1], F32)
            u8c = sb.tile([B, R], F32)
            rm8 = sb.tile([B, R], F32)
            effT = sb.tile([B, R], F32)
            b2 = sb.tile([P96, 1], F32)
            rpv = sb.tile([P96, 1], F32)
            scr96 = sb.tile([P96, R], F32)
            w96 = sb.tile([P96, 1], F32)
            c96 = sb.tile([P96, 1], F32)
            ridx = sb.tile([P96, 1], I32)

            stn = ps.tile([B, R], F32)
            eff96 = ps.tile([P96, R], F32)

            R8 = r8usel[:, 0:R]
            U8 = r8usel[:, R : R + 8]
            SEL96 = r8usel[:, R + 8 : R + 8 + P96]

            # repeat-APs over num_dets: k96[12n+j] = nd[n]; off96[12n+j] = nd[n-1]
            nd_rep8 = AP(nd_d, 0, [[1, B], [0, J], [1, 1]])
            nd_rep7 = AP(nd_d, 0, [[1, B - 1], [0, J], [1, 1]])
            nd_head7 = AP(nd_d, 0, [[1, B - 1], [1, 1]])

            # zeroed bases for the shifted loads
            nc.gpsimd.memset(off8i[:], 0)
            nc.gpsimd.memset(off96[:], 0)

            # loads, spread across the three DMA queues; critical ones first
            nc.sync.dma_start(out=k8i[:], in_=nd_d[:].rearrange("(p f) -> p f", f=1))
            nc.sync.dma_start(out=k96[:], in_=nd_rep8)
            nc.sync.dma_start(
                out=j96[:], in_=const_d[_O_J96 : _O_J96 + P96].rearrange(
                    "(p f) -> p f", f=1
                )
            )
            nc.sync.dma_start(out=x7[:, 5:6], in_=classes_d[:, 0:J])
            nc.scalar.dma_start(out=off8i[1:B, :], in_=nd_head7)
            nc.scalar.dma_start(out=off96[J:P96, :], in_=nd_rep7)
            nc.scalar.dma_start(out=x7[:, 1:5], in_=boxes_d[:, 0:J, :])
            nc.scalar.dma_start(out=x7[:, 6:7], in_=scores_d[:, 0:J])
            nc.gpsimd.dma_start(
                out=r8usel[:],
                in_=const_d[_O_R8:_O_R100].rearrange("(p f) -> p f", p=B),
            )
            nc.gpsimd.dma_start(
                out=r100[:], in_=const_d[_O_R100:CONST_LEN].rearrange(
                    "(p f) -> p f", p=P96
                )
            )
            nc.gpsimd.dma_start(
                out=x7[:, 0:1],
                in_=const_d[_O_VD : _O_VD + P96].rearrange("(p f) -> p f", f=1),
            )

            alu = mybir.AluOpType
            vec = nc.vector

            # zero-fill pass: scatter zeros to every out row, on the same
            # qPoolDynamic ring as the data scatter, so skipped rows are zero
            nc.gpsimd.memset(z7[:], 0.0)
            nc.gpsimd.iota(ridx0[:], pattern=[[1, 1]], base=0, channel_multiplier=1)
            zfill = nc.gpsimd.indirect_dma_start(
                out=out_d[:],
                out_offset=bass.IndirectOffsetOnAxis(ap=ridx0[:], axis=0),
                in_=z7[:],
                in_offset=None,
                bounds_check=R - 1,
                oob_is_err=False,
            )

            # batch coverage and last-writer masks
            vec.tensor_copy(off8f[:], off8i[:])                      # i32 -> f32
            vec.tensor_scalar(s8f[:], k8i[:], off8f[:], None, alu.add)
            vec.tensor_scalar(u8c[:], R8, off8f[:], None, alu.is_ge)
            vec.scalar_tensor_tensor(
                rm8[:], R8, s8f[:], u8c[:], alu.is_lt, alu.mult
            )
            nc.tensor.matmul(stn[:], U8, rm8[:], start=True, stop=True)
            vec.scalar_tensor_tensor(
                effT[:], stn[:], 0.0, rm8[:], alu.is_equal, alu.mult
            )
            nc.tensor.matmul(eff96[:], SEL96, effT[:], start=True, stop=True)

            # per-(n,j) scatter targets, gated to the winning writer only
            vec.tensor_scalar(b2[:], k96[:], j96[:], OOB, alu.is_le, alu.mult)
            vec.scalar_tensor_tensor(
                rpv[:], off96[:], j96[:], b2[:], alu.add, alu.add
            )
            vec.scalar_tensor_tensor(
                scr96[:], r100[:], rpv[:], eff96[:], alu.is_equal, alu.mult,
                accum_out=w96[:],
            )
            vec.tensor_scalar(c96[:], w96[:], -OOB, OOB, alu.mult, alu.add)
            vec.tensor_tensor(ridx[:], rpv[:], c96[:], alu.add)

            # winner-only scatter: destinations are unique, no ordering needed
            scat = nc.gpsimd.indirect_dma_start(
                out=out_d[:],
                out_offset=bass.IndirectOffsetOnAxis(ap=ridx[:], axis=0),
                in_=x7[:],
                in_offset=None,
                bounds_check=R - 1,
                oob_is_err=False,
            )
            # the zero-fill must fully land before the data scatter
            bass._add_dep_helper(
                scat.ins, zfill.ins, sync=True, reason="zero-fill before scatter"
            )

    nc.finalize()
    return nc


_CACHE: dict = {}


def _get_built():
    if "nc" not in _CACHE:
        _CACHE["nc"] = _build_nc()
        _CACHE["consts"] = _make_consts()
    return _CACHE["nc"], _CACHE["consts"]


def run(inputs: dict, trace: bool = False, **spmd_kwargs):
    """Run on all 8 cores with replicated inputs; returns (out, BassKernelResults)."""
    nc, consts = _get_built()
    in_map = {
        "num_dets": np.ascontiguousarray(inputs["num_dets"], dtype=np.int32),
        "boxes": np.ascontiguousarray(inputs["boxes"], dtype=np.float32),
        "scores": np.ascontiguousarray(inputs["scores"], dtype=np.float32),
        "classes": np.ascontiguousarray(inputs["classes"], dtype=np.float32),
        "consts": consts,
    }
    res = run_bass_kernel_spmd(
        nc,
        [dict(in_map) for _ in range(8)],
        core_ids=list(range(8)),
        trace=trace,
        **spmd_kwargs,
    )
    return res.results[0]["out"], res


def kernel(num_dets, boxes, scores, classes):
    out, _ = run(
        {"num_dets": num_dets, "boxes": boxes, "scores": scores, "classes": classes}
    )
    return out


# revision 11
# speedup vs baseline: 1.1312x; 1.1312x over previous
"""Trainium2 Bass kernel for nn_End2EndRVFixedOutput (nms_detection).

Reference semantics: out[100,7] starts at zeros; for n = 0..7 in order,
with off_n = (0 if n==0 else num_dets[n-1]) and k_n = num_dets[n],
rows [off_n, off_n+k_n) are overwritten with
[n, boxes[n,j,0:4], classes[n,j], scores[n,j]] for j = row-off_n.

num_dets < 12, so only the [:, :12] input slices matter and only out rows
0..21 can ever be written.  Device algorithm (per core, inputs replicated):

  1. x7[96,7] = [vd | boxes | classes | scores] for rows p = 12n+j is
     assembled by direct column DMAs straight from the full DRAM tensors.
  2. Winner masks are computed deterministically on DVE + PE:
        rm8[n,r]  = (off_n <= r < off_n+k_n)        # batch n covers row r
        stn[n,r]  = sum_{m>n} rm8[m,r]              # tiny suffix matmul
        effT[n,r] = rm8[n,r] * (stn[n,r]==0)        # n is the LAST writer
        EFF96     = SEL96 @ effT                    # broadcast to (n,j) rows
     (both matmuls in bf16 -- operands are exact small ints -- single pass)
     Per-row scatter targets and winner gating:
        rpv[p]  = off_n + j + 1e6*(j >= k_n)
        w96[p]  = sum_r (R100[p,r]==rpv[p]) * EFF96[p,r]   # fused accum_out
        ridx[p] = rpv[p] + 1e6*(1 - w96[p])
  3. A zero-fill indirect DMA writes zeros to all 100 rows, then the data
     scatter writes x7 rows to out[ridx] on the same qPoolDynamic ring.
     Gating makes destinations UNIQUE (at most one winner per row), so no
     reliance on DMA descriptor ordering; indices >= 1e6 are skipped via
     bounds_check.

All arithmetic is exact (masks are 0/1, indices are small ints), so the
output matches the reference bit-for-bit.  Every core runs the full
(tiny) computation; core 0's output is returned.
"""

import sys

import numpy as np

_TRN_REPO = "/opt/trn_rl_repo"
if _TRN_REPO not in sys.path:
    sys.path.insert(0, _TRN_REPO)

import ml_dtypes

import concourse.bacc as bacc
import concourse.bass as bass
import concourse.mybir as mybir
import concourse.tile as tile
from concourse.bass_types import AP
from concourse.bass_utils import run_bass_kernel_spmd

B = 8          # batches
N_FULL = 8192  # detections per batch in the full input
J = 12         # num_dets < 12, so only rows [:12] of each batch matter
R = 100        # fixed output rows
P96 = B * J    # 96 stacked (batch, j) rows
OOB = 1.0e6    # pushed past bounds_check so the scatter skips the row

F32 = mybir.dt.float32
BF16 = mybir.dt.bfloat16
I32 = mybir.dt.int32

# f32 constant blob: R8 [8,100] | CB96 [96,101] (R100|j96) | vd96 [96]
_O_R8 = 0
_O_CB96 = _O_R8 + 8 * R
_O_VD = _O_CB96 + P96 * (R + 1)
CONST_LEN = _O_VD + P96
# bf16 constant blob: U8 [8,8] | SEL96 [8,96] packed per-row as [8,104]
CONSTBF_LEN = 8 * (8 + P96)


def _make_consts():
    p = np.arange(P96)
    m = np.arange(B)
    r8 = np.tile(np.arange(R, dtype=np.float32)[None, :], (B, 1))        # [8,100]
    r100 = np.tile(np.arange(R, dtype=np.float32)[None, :], (P96, 1))    # [96,100]
    j96 = (p % J).astype(np.float32)[:, None]                            # [96,1]
    cb96 = np.concatenate([r100, j96], axis=1)                           # [96,101]
    vd96 = (p // J).astype(np.float32)                                   # [96]
    blob = np.concatenate([r8.ravel(), cb96.ravel(), vd96]).astype(np.float32)
    assert blob.shape == (CONST_LEN,)
    u8 = (m[:, None] > m[None, :]).astype(np.float32)                    # [8,8]
    sel96 = (m[:, None] == p[None, :] // J).astype(np.float32)           # [8,96]
    blobbf = (
        np.concatenate([u8, sel96], axis=1).ravel().astype(ml_dtypes.bfloat16)
    )
    assert blobbf.shape == (CONSTBF_LEN,)
    return np.ascontiguousarray(blob), np.ascontiguousarray(blobbf)


def _build_nc() -> bass.Bass:
    nc = bacc.Bacc(None, target_bir_lowering=False)
    nd_d = nc.dram_tensor("num_dets", [B], I32, kind="ExternalInput")
    boxes_d = nc.dram_tensor("boxes", [B, N_FULL, 4], F32, kind="ExternalInput")
    scores_d = nc.dram_tensor("scores", [B, N_FULL], F32, kind="ExternalInput")
    classes_d = nc.dram_tensor("classes", [B, N_FULL], F32, kind="ExternalInput")
    const_d = nc.dram_tensor("consts", [CONST_LEN], F32, kind="ExternalInput")
    constbf_d = nc.dram_tensor("constsbf", [CONSTBF_LEN], BF16, kind="ExternalInput")
    out_d = nc.dram_tensor("out", [R, 7], F32, kind="ExternalOutput")

    with tile.TileContext(nc) as tc:
        with (
            tc.tile_pool(name="sb", bufs=1) as sb,
            tc.tile_pool(name="ps", bufs=1, space=bass.MemorySpace.PSUM) as ps,
        ):
            ndi = sb.tile([B, 1], I32)
            k96 = sb.tile([P96, 1], I32)
            off96 = sb.tile([P96, 1], I32)
            r8t = sb.tile([B, R], F32)
            cb96 = sb.tile([P96, R + 1], F32)
            usel = sb.tile([B, 8 + P96], BF16)
            x7 = sb.tile([P96, 7], F32)
            z7 = sb.tile([R, 7], F32)
            ridx0 = sb.tile([R, 1], I32)

            k32 = sb.tile([32, 1], F32)
            off32 = sb.tile([32, 1], F32)
            s8f = sb.tile([B, 1], F32)
            u8c = sb.tile([B, R], F32)
            rm8 = sb.tile([B, R], BF16)
            effT = sb.tile([B, R], BF16)
            b2 = sb.tile([P96, 1], F32)
            rpv = sb.tile([P96, 1], F32)
            scr96 = sb.tile([P96, R], F32)
            w96 = sb.tile([P96, 1], F32)
            c96 = sb.tile([P96, 1], F32)
            ridx = sb.tile([P96, 1], I32)

            stn = ps.tile([B, R], F32)
            eff96 = ps.tile([P96, R], F32)

            U8 = usel[:, 0:8]
            SEL96 = usel[:, 8 : 8 + P96]
            R100 = cb96[:, 0:R]
            J96 = cb96[:, R : R + 1]

            # repeat-APs over num_dets: k96[12n+j] = nd[n]; off96[12n+j] = nd[n-1]
            nd_rep8 = AP(nd_d, 0, [[1, B], [0, J], [1, 1]])
            nd_rep7 = AP(nd_d, 0, [[1, B - 1], [0, J], [1, 1]])

            nc.gpsimd.memset(off96[:], 0)
            nc.gpsimd.memset(k32[:], 0.0)
            nc.gpsimd.memset(z7[:], 0.0)
            nc.gpsimd.iota(ridx0[:], pattern=[[1, 1]], base=0, channel_multiplier=1)

            # zero-fill pass: scatter zeros to every out row, on the same
            # qPoolDynamic ring as the data scatter, so skipped rows are zero
            zfill = nc.gpsimd.indirect_dma_start(
                out=out_d[:],
                out_offset=bass.IndirectOffsetOnAxis(ap=ridx0[:], axis=0),
                in_=z7[:],
                in_offset=None,
                bounds_check=R - 1,
                oob_is_err=False,
            )

            # loads, spread across the three DMA queues; critical ones first
            nc.sync.dma_start(out=ndi[:], in_=nd_d[:].rearrange("(p f) -> p f", f=1))
            nc.sync.dma_start(out=k96[:], in_=nd_rep8)
            nc.sync.dma_start(out=off96[J:P96, :], in_=nd_rep7)
            nc.sync.dma_start(out=x7[:, 5:6], in_=classes_d[:, 0:J])
            nc.scalar.dma_start(
                out=r8t[:], in_=const_d[_O_R8:_O_CB96].rearrange("(p f) -> p f", p=B)
            )
            nc.scalar.dma_start(out=x7[:, 1:5], in_=boxes_d[:, 0:J, :])
            nc.scalar.dma_start(out=x7[:, 6:7], in_=scores_d[:, 0:J])
            nc.gpsimd.dma_start(out=usel[:], in_=constbf_d[:].rearrange(
                "(p f) -> p f", p=B
            ))
            nc.gpsimd.dma_start(
                out=cb96[:], in_=const_d[_O_CB96:_O_VD].rearrange("(p f) -> p f", p=P96)
            )
            nc.gpsimd.dma_start(
                out=x7[:, 0:1],
                in_=const_d[_O_VD : _O_VD + P96].rearrange("(p f) -> p f", f=1),
            )

            alu = mybir.AluOpType
            vec = nc.vector

            # k32[0:8] = float(num_dets); off32[n] = k32[n-1] via partition shift
            vec.tensor_copy(k32[0:B, :], ndi[:])
            vec.stream_shuffle(off32[:], k32[:], mask=[31] + list(range(31)))
            # batch coverage and last-writer masks
            vec.tensor_tensor(s8f[:], k32[0:B, :], off32[0:B, :], alu.add)
            vec.tensor_scalar(u8c[:], r8t[:], off32[0:B, :], None, alu.is_ge)
            vec.scalar_tensor_tensor(
                rm8[:], r8t[:], s8f[:], u8c[:], alu.is_lt, alu.mult
            )
            nc.tensor.matmul(stn[:], U8, rm8[:], start=True, stop=True)
            vec.scalar_tensor_tensor(
                effT[:], stn[:], 0.0, rm8[:], alu.is_equal, alu.mult
            )
            nc.tensor.matmul(eff96[:], SEL96, effT[:], start=True, stop=True)

            # per-(n,j) scatter targets, gated to the winning writer only
            vec.tensor_scalar(b2[:], k96[:], J96, OOB, alu.is_le, alu.mult)
            vec.scalar_tensor_tensor(
                rpv[:], off96[:], J96, b2[:], alu.add, alu.add
            )
            vec.scalar_tensor_tensor(
                scr96[:], R100, rpv[:], eff96[:], alu.is_equal, alu.mult,
                accum_out=w96[:],
            )
            vec.tensor_scalar(c96[:], w96[:], -OOB, OOB, alu.mult, alu.add)
            vec.tensor_tensor(ridx[:], rpv[:], c96[:], alu.add)

            # winner-only scatter: destinations are unique, no ordering needed
            scat = nc.gpsimd.indirect_dma_start(
                out=out_d[:],
                out_offset=bass.IndirectOffsetOnAxis(ap=ridx[:], axis=0),
                in_=x7[:],
                in_offset=None,
                bounds_check=R - 1,
                oob_is_err=False,
            )
            # the zero-fill must fully land before the data scatter
            bass._add_dep_helper(
                scat.ins, zfill.ins, sync=True, reason="zero-fill before scatter"
            )

    nc.finalize()
    return nc


_CACHE: dict = {}


def _get_built():
    if "nc" not in _CACHE:
        _CACHE["nc"] = _build_nc()
        _CACHE["consts"] = _make_consts()
    return _CACHE["nc"], _CACHE["consts"]


def run(inputs: dict, trace: bool = False, **spmd_kwargs):
    """Run on all 8 cores with replicated inputs; returns (out, BassKernelResults)."""
    nc, (consts, constsbf) = _get_built()
    in_map = {
        "num_dets": np.ascontiguousarray(inputs["num_dets"], dtype=np.int32),
        "boxes": np.ascontiguousarray(inputs["boxes"], dtype=np.float32),
        "scores": np.ascontiguousarray(inputs["scores"], dtype=np.float32),
        "classes": np.ascontiguousarray(inputs["classes"], dtype=np.float32),
        "consts": consts,
        "constsbf": constsbf,
    }
    res = run_bass_kernel_spmd(
        nc,
        [dict(in_map) for _ in range(8)],
        core_ids=list(range(8)),
        trace=trace,
        **spmd_kwargs,
    )
    return res.results[0]["out"], res


def kernel(num_dets, boxes, scores, classes):
    out, _ = run(
        {"num_dets": num_dets, "boxes": boxes, "scores": scores, "classes": classes}
    )
    return out


# revision 16
# speedup vs baseline: 1.2522x; 1.1069x over previous
"""Trainium2 Bass kernel for nn_End2EndRVFixedOutput (nms_detection).

Reference semantics: out[100,7] starts at zeros; for n = 0..7 in order,
with off_n = (0 if n==0 else num_dets[n-1]) and k_n = num_dets[n],
rows [off_n, off_n+k_n) are overwritten with
[n, boxes[n,j,0:4], classes[n,j], scores[n,j]] for j = row-off_n.

num_dets < 12, so only the [:, :12] input slices matter and only out rows
0..21 can ever be written.  Device algorithm (per core, inputs replicated):

  1. x7[96,7] = [vd | boxes | classes | scores] for rows p = 12n+j is
     assembled by direct column DMAs straight from the full DRAM tensors.
  2. Winner masks are computed deterministically on DVE + PE:
        rm8[n,r]  = (off_n <= r < off_n+k_n)        # batch n covers row r
        stn[n,r]  = sum_{m>n} rm8[m,r]              # tiny suffix matmul
        effT[n,r] = rm8[n,r] * (stn[n,r]==0)        # n is the LAST writer
        EFF96     = SEL96 @ effT                    # broadcast to (n,j) rows
     (both matmuls in bf16 -- operands are exact small ints -- single pass)
     Per-row scatter targets and winner gating:
        rpv[p]  = off_n + j + 1e6*(j >= k_n)
        w96[p]  = sum_r (R100[p,r]==rpv[p]) * EFF96[p,r]   # fused accum_out
        ridx[p] = rpv[p] + 1e6*(1 - w96[p])
  3. A zero-fill indirect DMA writes zeros to all 100 rows, then the data
     scatter writes x7 rows to out[ridx] on the same qPoolDynamic ring.
     Gating makes destinations UNIQUE (at most one winner per row), so no
     reliance on DMA descriptor ordering; indices >= 1e6 are skipped via
     bounds_check.

All arithmetic is exact (masks are 0/1, indices are small ints), so the
output matches the reference bit-for-bit.  Every core runs the full
(tiny) computation; core 0's output is returned.
"""

import sys

import numpy as np

_TRN_REPO = "/opt/trn_rl_repo"
if _TRN_REPO not in sys.path:
    sys.path.insert(0, _TRN_REPO)

import ml_dtypes

import concourse.bacc as bacc
import concourse.bass as bass
import concourse.mybir as mybir
import concourse.tile as tile
from concourse.bass_types import AP
from concourse.bass_utils import run_bass_kernel_spmd

B = 8          # batches
N_FULL = 8192  # detections per batch in the full input
J = 12         # num_dets < 12, so only rows [:12] of each batch matter
R = 100        # fixed output rows
P96 = B * J    # 96 stacked (batch, j) rows
OOB = 1.0e6    # pushed past bounds_check so the scatter skips the row

F32 = mybir.dt.float32
BF16 = mybir.dt.bfloat16
I32 = mybir.dt.int32

# f32 constant blob: CB96 [96,102] = R100 | j96 | vd96
CONST_LEN = P96 * (R + 2)
# bf16 constant blob: U8 [8,8] | SEL96 [8,96] packed per-row as [8,104]
CONSTBF_LEN = 8 * (8 + P96)


def _make_consts():
    p = np.arange(P96)
    m = np.arange(B)
    r100 = np.tile(np.arange(R, dtype=np.float32)[None, :], (P96, 1))    # [96,100]
    j96 = (p % J).astype(np.float32)[:, None]                            # [96,1]
    vd96 = (p // J).astype(np.float32)[:, None]                          # [96,1]
    blob = np.concatenate([r100, j96, vd96], axis=1).ravel().astype(np.float32)
    assert blob.shape == (CONST_LEN,)
    u8 = (m[:, None] > m[None, :]).astype(np.float32)                    # [8,8]
    sel96 = (m[:, None] == p[None, :] // J).astype(np.float32)           # [8,96]
    blobbf = (
        np.concatenate([u8, sel96], axis=1).ravel().astype(ml_dtypes.bfloat16)
    )
    assert blobbf.shape == (CONSTBF_LEN,)
    return np.ascontiguousarray(blob), np.ascontiguousarray(blobbf)


def _build_nc() -> bass.Bass:
    nc = bacc.Bacc(None, target_bir_lowering=False)
    nd_d = nc.dram_tensor("num_dets", [B], I32, kind="ExternalInput")
    boxes_d = nc.dram_tensor("boxes", [B, N_FULL, 4], F32, kind="ExternalInput")
    scores_d = nc.dram_tensor("scores", [B, N_FULL], F32, kind="ExternalInput")
    classes_d = nc.dram_tensor("classes", [B, N_FULL], F32, kind="ExternalInput")
    const_d = nc.dram_tensor("consts", [CONST_LEN], F32, kind="ExternalInput")
    constbf_d = nc.dram_tensor("constsbf", [CONSTBF_LEN], BF16, kind="ExternalInput")
    out_d = nc.dram_tensor("out", [R, 7], F32, kind="ExternalOutput")

    with tile.TileContext(nc) as tc:
        with (
            tc.tile_pool(name="sb", bufs=1) as sb,
            tc.tile_pool(name="ps", bufs=1, space=bass.MemorySpace.PSUM) as ps,
        ):
            ndi = sb.tile([B, 1], I32)
            k96 = sb.tile([P96, 1], I32)
            off96 = sb.tile([P96, 1], I32)
            cb96 = sb.tile([P96, R + 2], F32)
            usel = sb.tile([B, 8 + P96], BF16)
            x7 = sb.tile([P96, 7], F32)
            z7 = sb.tile([R, 7], F32)
            ridx0 = sb.tile([R, 1], I32)

            k32 = sb.tile([32, 1], F32)
            off32 = sb.tile([32, 1], F32)
            s8f = sb.tile([B, 1], F32)
            u8c = sb.tile([B, R], F32)
            rm8 = sb.tile([B, R], BF16)
            effT = sb.tile([B, R], BF16)
            b2 = sb.tile([P96, 1], F32)
            rpv = sb.tile([P96, 1], F32)
            scr96 = sb.tile([P96, R], F32)
            w96 = sb.tile([P96, 1], F32)
            c96 = sb.tile([P96, 1], F32)
            ridx = sb.tile([P96, 1], I32)

            stn = ps.tile([B, R], F32)
            eff96 = ps.tile([P96, R], F32)

            U8 = usel[:, 0:8]
            SEL96 = usel[:, 8 : 8 + P96]
            R100 = cb96[:, 0:R]
            R8 = cb96[0:B, 0:R]
            J96 = cb96[:, R : R + 1]
            VD96 = cb96[:, R + 1 : R + 2]

            # repeat-APs over num_dets: k96[12n+j] = nd[n]; off96[12n+j] = nd[n-1]
            nd_rep8 = AP(nd_d, 0, [[1, B], [0, J], [1, 1]])
            nd_rep7 = AP(nd_d, 0, [[1, B - 1], [0, J], [1, 1]])

            nc.gpsimd.memset(off96[:], 0)
            nc.gpsimd.memset(k32[:], 0.0)
            nc.gpsimd.memset(z7[:], 0.0)
            nc.gpsimd.iota(ridx0[:], pattern=[[1, 1]], base=0, channel_multiplier=1)

            # zero-fill pass: scatter zeros to every out row, on the same
            # qPoolDynamic ring as the data scatter, so skipped rows are zero
            zfill = nc.gpsimd.indirect_dma_start(
                out=out_d[:],
                out_offset=bass.IndirectOffsetOnAxis(ap=ridx0[:], axis=0),
                in_=z7[:],
                in_offset=None,
                bounds_check=R - 1,
                oob_is_err=False,
            )

            # loads on the two HWDGE queues only (gpsimd is kept free for the
            # indirect scatters); critical ones first
            nc.sync.dma_start(out=ndi[:], in_=nd_d[:].rearrange("(p f) -> p f", f=1))
            nc.sync.dma_start(out=k96[:], in_=nd_rep8)
            nc.sync.dma_start(out=off96[J:P96, :], in_=nd_rep7)
            nc.sync.dma_start(out=x7[:, 5:6], in_=classes_d[:, 0:J])
            nc.scalar.dma_start(
                out=cb96[:], in_=const_d[:].rearrange("(p f) -> p f", p=P96)
            )
            nc.scalar.dma_start(out=usel[:], in_=constbf_d[:].rearrange(
                "(p f) -> p f", p=B
            ))
            nc.scalar.dma_start(out=x7[:, 1:5], in_=boxes_d[:, 0:J, :])
            nc.scalar.dma_start(out=x7[:, 6:7], in_=scores_d[:, 0:J])

            alu = mybir.AluOpType
            vec = nc.vector

            # k32[0:8] = float(num_dets); off32[n] = k32[n-1] via partition shift
            vec.tensor_copy(k32[0:B, :], ndi[:])
            vec.stream_shuffle(off32[:], k32[:], mask=[31] + list(range(31)))
            # vd column of x7 comes straight out of the const tile
            vec.tensor_copy(x7[:, 0:1], VD96)
            # batch coverage and last-writer masks
            vec.tensor_tensor(s8f[:], k32[0:B, :], off32[0:B, :], alu.add)
            vec.tensor_scalar(u8c[:], R8, off32[0:B, :], None, alu.is_ge)
            vec.scalar_tensor_tensor(
                rm8[:], R8, s8f[:], u8c[:], alu.is_lt, alu.mult
            )
            nc.tensor.matmul(stn[:], U8, rm8[:], start=True, stop=True)
            vec.scalar_tensor_tensor(
                effT[:], stn[:], 0.0, rm8[:], alu.is_equal, alu.mult
            )
            nc.tensor.matmul(eff96[:], SEL96, effT[:], start=True, stop=True)

            # per-(n,j) scatter targets, gated to the winning writer only
            vec.tensor_scalar(b2[:], k96[:], J96, OOB, alu.is_le, alu.mult)
            vec.scalar_tensor_tensor(
                rpv[:], off96[:], J96, b2[:], alu.add, alu.add
            )
            vec.scalar_tensor_tensor(
                scr96[:], R100, rpv[:], eff96[:], alu.is_equal, alu.mult,
                accum_out=w96[:],
            )
            vec.tensor_scalar(c96[:], w96[:], -OOB, OOB, alu.mult, alu.add)
            vec.tensor_tensor(ridx[:], rpv[:], c96[:], alu.add)

            # winner-only scatter: destinations are unique, no ordering needed
            scat = nc.gpsimd.indirect_dma_start(
                out=out_d[:],
                out_offset=bass.IndirectOffsetOnAxis(ap=ridx[:], axis=0),
                in_=x7[:],
                in_offset=None,
                bounds_check=R - 1,
                oob_is_err=False,
            )
            # the zero-fill must fully land before the data scatter
            bass._add_dep_helper(
                scat.ins, zfill.ins, sync=True, reason="zero-fill before scatter"
            )

    nc.finalize()
    return nc


_CACHE: dict = {}


def _get_built():
    if "nc" not in _CACHE:
        _CACHE["nc"] = _build_nc()
        _CACHE["consts"] = _make_consts()
    return _CACHE["nc"], _CACHE["consts"]


def run(inputs: dict, trace: bool = False, **spmd_kwargs):
    """Run on all 8 cores with replicated inputs; returns (out, BassKernelResults)."""
    nc, (consts, constsbf) = _get_built()
    in_map = {
        "num_dets": np.ascontiguousarray(inputs["num_dets"], dtype=np.int32),
        "boxes": np.ascontiguousarray(inputs["boxes"], dtype=np.float32),
        "scores": np.ascontiguousarray(inputs["scores"], dtype=np.float32),
        "classes": np.ascontiguousarray(inputs["classes"], dtype=np.float32),
        "consts": consts,
        "constsbf": constsbf,
    }
    res = run_bass_kernel_spmd(
        nc,
        [dict(in_map) for _ in range(8)],
        core_ids=list(range(8)),
        trace=trace,
        **spmd_kwargs,
    )
    return res.results[0]["out"], res


def kernel(num_dets, boxes, scores, classes):
    out, _ = run(
        {"num_dets": num_dets, "boxes": boxes, "scores": scores, "classes": classes}
    )
    return out
